# revision 1
# baseline (speedup 1.0000x reference)
"""Distributed Trainium2 Bass kernel for pre-LN multi-head attention.

Reference computation (per batch b of 2, seq n=2048, dim=1024, 16 heads x 64):
    xn = LayerNorm(x) * gamma + beta
    q, k = split(xn @ W_qk); v = xn @ W_v
    out = softmax(q k^T / 8) v  (per head)
    y = out @ W_out + b_out

Sharding: 8 cores = 2 batch groups x 4 sequence quarters. Core i owns batch
g=i//4, query tokens [qq*512, (qq+1)*512) with qq=i%4. Each core computes
LN + Q/K/V projections for its own 512 tokens, AllGathers K^T and V across
its 4-core group (full 2048-token K/V per batch), runs attention for its 512
queries over all 2048 keys (all 16 heads), and applies the output projection
with the full W_out — so the final output needs no inter-core reduction.
Host assembles the 8 per-core [1024, 512] y^T shards into [2, 2048, 1024].

Optimizations vs the v1 kernel:
- Host packs W_qk into separate K-half / Q-half tensors and gamma/beta/b_out
  into one [128, 24] block, so every weight reaches SBUF in one large
  contiguous-line DMA (no 4-byte gather packets).
- Input DMA order = consumption order: x first (feeds LN), then the K-half
  and W_v (feed the collective payload), Q-half and W_out last; every DMA
  stays <=256KB (bigger ones stall the HWDGE ring and starve the queue).
- A short dummy-matmul chain warms the PE clock before real work arrives.
- All accumulation chains (K/V/Q projections, output projection) run 2-4
  chains interleaved across PSUM banks: back-to-back matmuls into one
  accumulator serialize at ~2x the cost of independent ones.
- The gather is split into 6 subgroups [2,2,4,4,2,2]: staging is ready at
  ~44us, the first payload lands right after the fixed ~60us CC mesh
  startup, and the last payloads are small so the tail is short. Loads for
  the next subgroup are emitted only after the current one's compute so a
  gather-blocked DMA can never stall compute queues behind it.
- Gathered V is stored per key-chunk as [ones(64) | V_h0..h15], so every
  head's AV stationary is a uniform strided AP [ones | V_h] (colsum rows on
  PSUM partitions 0:64, data on 64:128 for all heads) and each (group,
  quarter)'s V lands with one 3-level DMA with 512B lines; the K^T loads are
  merged to two DMAs per (group, quarter). 12 DMAs per group vs 48 before.
"""
import sys
import types

sys.path.insert(0, "/opt/trn_rl_repo")

# Register the NTFF profile hook that trn_boot skips when the image's antenv
# lacks axon_hooks, so run_bass_kernel_spmd(trace=True) can report exec time.
if "antenv.axon_hooks" not in sys.modules:
    try:
        from trn_agent_boot.trn_boot import _ntff_profile_via_ctypes

        _hook = _ntff_profile_via_ctypes("/opt/axon/libaxon_pjrt.so")
    except Exception:
        _hook = None
    _mod = types.ModuleType("antenv.axon_hooks")
    _mod.get_axon_ntff_profile_hook = lambda: _hook
    _mod.set_axon_ntff_profile_hook = lambda h: None
    sys.modules["antenv.axon_hooks"] = _mod

from contextlib import ExitStack

import ml_dtypes
import numpy as np
import concourse.bass as bass
import concourse.tile as tile
from concourse import bacc, mybir
from concourse.bass_utils import run_bass_kernel_spmd
from concourse.masks import make_identity

F32 = mybir.dt.float32
BF16 = mybir.dt.bfloat16
AF = mybir.ActivationFunctionType
ALU = mybir.AluOpType

B, N, DIM = 2, 2048, 1024
HEADS, DH = 16, 64
INNER = HEADS * DH  # 1024
SCALE = DH**-0.5
EPS = 1e-5

NCORES = 8
GROUP = 4          # cores per batch group (sequence quarters)
NQ = N // GROUP    # 512 local query tokens per core
DC = DIM // 128    # 8 dim chunks
KCH = N // 128     # 16 key chunks of 128 tokens
KPAIR = KCH // 2   # exp batches of 2 key chunks

MMDT = BF16        # matmul operand storage dtype

REPLICA_GROUPS = [[0, 1, 2, 3], [4, 5, 6, 7]]

# Gather subgroups: 2-head groups at both ends — the first payload lands
# soon after the mesh barrier so the exp stream starts early, and the last
# payloads are small so the post-chain tail is short; 4-head groups in the
# middle keep the per-gather fixed cost amortized.
SG = [[0, 1], [2, 3], [4, 5, 6, 7], [8, 9, 10, 11], [12, 13], [14, 15]]


def sg_klen(hh):
    return len(hh) * 64 * NQ


def sg_len(hh):
    return 2 * len(hh) * 64 * NQ

VSTR = 1600              # per-key-chunk vones stride: 8x[ones64|V_2c|V_2c+1] + ones64


def build_nc():
    nc = bacc.Bacc(num_devices=NCORES)

    x = nc.dram_tensor("x", [NQ, DIM], F32, kind="ExternalInput")
    gbb = nc.dram_tensor("gbb", [128, 24], F32, kind="ExternalInput")
    wqk_k = nc.dram_tensor("wqk_k", [DIM, INNER], MMDT, kind="ExternalInput")
    wqk_q = nc.dram_tensor("wqk_q", [DIM, INNER], MMDT, kind="ExternalInput")
    w_v = nc.dram_tensor("W_v", [DIM, INNER], MMDT, kind="ExternalInput")
    w_out = nc.dram_tensor("W_out", [INNER, DIM], MMDT, kind="ExternalInput")
    out = nc.dram_tensor("out", [DIM, NQ], MMDT, kind="ExternalOutput")

    with tile.TileContext(nc) as tc, ExitStack() as ctx:
        pool = lambda name, bufs, **kw: ctx.enter_context(
            tc.tile_pool(name=name, bufs=bufs, **kw)
        )
        consts = pool("consts", 1)
        dram = pool("dram", 1, space="DRAM")
        qt_pool = pool("qt", 1)
        att_pool = pool("att", 1)
        small = pool("small", 8)
        stage = pool("stage", 4)
        pp = pool("pp", 2, space="PSUM")      # proj / outproj accumulators

        # ---- constants ---------------------------------------------------
        gbb_t = consts.tile([128, 24], F32)   # [gamma | beta | b_out] per c
        nc.sync.dma_start(gbb_t[:], gbb[:, :])
        ident = consts.tile([128, 128], MMDT)
        make_identity(nc, ident[:])
        eps_sb = consts.tile([128, 1], F32)
        nc.vector.memset(eps_sb[:], EPS)
        # PE warmup: ramp the clock while input DMAs are in flight.
        wps = pp.tile([128, 512], F32, tag="acc", name="warmup")
        for i in range(24):
            nc.tensor.matmul(
                wps[:, 0:128], ident[:], ident[:], start=(i == 0), stop=(i == 23)
            )

        cc_ins = []
        cc_outs = []
        for s, hh in enumerate(SG):
            cc_i = dram.tile([sg_len(hh)], MMDT, name=f"cc_in{s}")
            cc_o = dram.tile([GROUP * sg_len(hh)], MMDT, name=f"cc_out{s}")
            cc_ins.append(cc_i)
            cc_outs.append(cc_o)

        # Q^T duplicated per head: head h at cols h*512, rows 0:64 and
        # 64:128 both hold Q_h^T (so S^T matmuls contract over K=128,
        # computing 2*S — folded into the exp scale; K=64 matmuls were
        # observed to hold the HAM clock gate at 1.2 GHz).
        q_t = qt_pool.tile([128, HEADS * NQ], MMDT)
        # attention output^T [1024, 512], chunk c holds heads 2c, 2c+1
        att_t = att_pool.tile([128, DC * NQ], MMDT)

        with ExitStack() as proj_ctx:
            ppool = lambda name, bufs, **kw: proj_ctx.enter_context(
                tc.tile_pool(name=name, bufs=bufs, **kw)
            )
            ptr = ppool("ptr", 2, space="PSUM")  # transpose targets
            pkv = ppool("pkv", 2, space="PSUM")  # second accumulation chain
            xw = ppool("xw", 1)
            x_sb = xw.tile([128, GROUP * DIM], F32)
            xn_nat = xw.tile([128, GROUP * DIM], MMDT)
            xnt = xw.tile([128, DC * NQ], MMDT)
            wk_sb = xw.tile([128, DC * INNER], MMDT)
            wv_sb = xw.tile([128, DC * INNER], MMDT)
            wq_sb = xw.tile([128, DC * INNER], MMDT)

            # Input loads in consumption order, all on the Sync queue.
            # 256KB per DMA: bigger single DMAs stall the HWDGE ring and
            # starve packet dispatch for everything queued behind them.
            for t in range(GROUP):
                nc.sync.dma_start(
                    x_sb[:, t * DIM : (t + 1) * DIM],
                    x[t * 128 : (t + 1) * 128, :],
                )
            for c in range(DC):
                nc.sync.dma_start(
                    wk_sb[:, c * INNER : (c + 1) * INNER],
                    wqk_k[c * 128 : (c + 1) * 128, :],
                )
            for c in range(DC):
                nc.sync.dma_start(
                    wv_sb[:, c * INNER : (c + 1) * INNER],
                    w_v[c * 128 : (c + 1) * 128, :],
                )
            for c in range(DC):
                nc.sync.dma_start(
                    wq_sb[:, c * INNER : (c + 1) * INNER],
                    wqk_q[c * 128 : (c + 1) * 128, :],
                )

            # ---- LayerNorm on the 4 local token chunks ------------------
            for t in range(GROUP):
                xt = x_sb[:, t * DIM : (t + 1) * DIM]
                xg = xt.rearrange("p (n s) -> p n s", s=512)
                stats = small.tile([128, 2, 6], F32)
                for sgi in range(2):
                    nc.vector.bn_stats(stats[:, sgi, :], xg[:, sgi, :])
                mv = small.tile([128, 2], F32)
                nc.vector.bn_aggr(mv[:], stats[:])
                rstd = small.tile([128, 1], F32)
                nc.scalar.activation(rstd[:], mv[:, 1:2], AF.Sqrt, bias=eps_sb[:])
                nc.vector.reciprocal(rstd[:], rstd[:])
                nc.vector.tensor_scalar(
                    out=xn_nat[:, t * DIM : (t + 1) * DIM],
                    in0=xt,
                    scalar1=mv[:, 0:1],
                    scalar2=rstd[:],
                    op0=ALU.subtract,
                    op1=ALU.mult,
                )

            # ---- transpose xn to [dim, tokens], fusing gamma/beta -------
            # split the scale/cast between Vector and Scalar engines
            for c in range(DC):
                for t in range(GROUP):
                    pt = ptr.tile([128, 128], MMDT)
                    nc.tensor.transpose(
                        pt[:],
                        xn_nat[:, t * DIM + c * 128 : t * DIM + (c + 1) * 128],
                        ident[:],
                    )
                    dst = xnt[:, c * NQ + t * 128 : c * NQ + (t + 1) * 128]
                    if t % 2 == 0:
                        nc.vector.tensor_scalar(
                            out=dst,
                            in0=pt[:],
                            scalar1=gbb_t[:, c : c + 1],
                            scalar2=gbb_t[:, 8 + c : 9 + c],
                            op0=ALU.mult,
                            op1=ALU.add,
                        )
                    else:
                        nc.scalar.activation(
                            dst,
                            pt[:],
                            AF.Identity,
                            bias=gbb_t[:, 8 + c : 9 + c],
                            scale=gbb_t[:, c : c + 1],
                        )

            # ---- per subgroup: K^T + V projections, then its AllGather ---
            # Two accumulation chains run interleaved (separate PSUM banks):
            # back-to-back matmuls into one accumulator serialize (~720ns
            # each), interleaved independent chains pipeline (~2x faster).
            def k_chains(ms):
                pqs = [
                    (pp if i == 0 else pkv).tile(
                        [128, 512],
                        F32,
                        tag="acc" if i == 0 else "kvacc",
                        name=f"pk{i}",
                    )
                    for i in range(len(ms))
                ]
                for c in range(DC):
                    for i, m in enumerate(ms):
                        nc.tensor.matmul(
                            pqs[i][:],
                            wk_sb[:, c * INNER + m * 128 : c * INNER + (m + 1) * 128],
                            xnt[:, c * NQ : (c + 1) * NQ],
                            start=(c == 0),
                            stop=(c == DC - 1),
                        )
                return pqs

            def stage_k(s, hh, mi, pq):
                kst = stage.tile([128, 512], MMDT, tag="stg")
                nc.scalar.copy(kst[:], pq[:])
                koff = mi * 128 * NQ
                nc.sync.dma_start(
                    cc_ins[s][koff : koff + 128 * NQ].rearrange("(p f) -> p f", f=NQ),
                    kst[:],
                )

            def stage_v(s, hh):
                vw = len(hh) * 64  # V columns for this subgroup
                for tp in range(2):
                    pvs = [
                        (pp if ti == 0 else pkv).tile(
                            [128, 512],
                            F32,
                            tag="acc" if ti == 0 else "kvacc",
                            name=f"pv{ti}",
                        )
                        for ti in range(2)
                    ]
                    for c in range(DC):
                        for ti in range(2):
                            t = 2 * tp + ti
                            nc.tensor.matmul(
                                pvs[ti][:, 0:vw],
                                xnt[:, c * NQ + t * 128 : c * NQ + (t + 1) * 128],
                                wv_sb[
                                    :,
                                    c * INNER + hh[0] * 64 : c * INNER
                                    + (hh[0] + len(hh)) * 64,
                                ],
                                start=(c == 0),
                                stop=(c == DC - 1),
                            )
                    for ti in range(2):
                        t = 2 * tp + ti
                        vst = stage.tile([128, 512], MMDT, tag="stg")
                        nc.vector.tensor_copy(vst[:, 0:vw], pvs[ti][:, 0:vw])
                        voff = sg_klen(hh) + t * 128 * vw
                        nc.sync.dma_start(
                            cc_ins[s][voff : voff + 128 * vw].rearrange(
                                "(p f) -> p f", f=vw
                            ),
                            vst[:, 0:vw],
                        )

            def gather(s):
                nc.gpsimd.collective_compute(
                    "AllGather",
                    ALU.bypass,
                    replica_groups=REPLICA_GROUPS,
                    ins=[cc_ins[s][:].opt()],
                    outs=[cc_outs[s][:].opt()],
                )

            # 2-head subgroups pair up their K chains for interleaving;
            # each subgroup stages and gathers as soon as it is ready.
            def stage_pair2(sa, sb):
                pq01 = k_chains([SG[sa][0] // 2, SG[sb][0] // 2])
                stage_k(sa, SG[sa], 0, pq01[0])
                stage_v(sa, SG[sa])
                gather(sa)
                stage_k(sb, SG[sb], 0, pq01[1])
                stage_v(sb, SG[sb])
                gather(sb)

            stage_pair2(0, 1)
            for s in (2, 3):
                hh = SG[s]
                mb = hh[0] // 2
                pqs = k_chains([mb, mb + 1])
                for mi in range(2):
                    stage_k(s, hh, mi, pqs[mi])
                stage_v(s, hh)
                gather(s)
            stage_pair2(4, 5)

            # ---- Q^T projection, overlaps the AllGathers ----------------
            for mp in range(DC // 2):
                pq0 = pp.tile([128, 512], F32, tag="acc")
                pq1 = pkv.tile([128, 512], F32, tag="kvacc")
                pqs = [pq0, pq1]
                for c in range(DC):
                    for mi in range(2):
                        m = 2 * mp + mi
                        nc.tensor.matmul(
                            pqs[mi][:],
                            wq_sb[:, c * INNER + m * 128 : c * INNER + (m + 1) * 128],
                            xnt[:, c * NQ : (c + 1) * NQ],
                            start=(c == 0),
                            stop=(c == DC - 1),
                        )
                for mi in range(2):
                    m = 2 * mp + mi
                    for lh in range(2):
                        h_abs = 2 * m + lh
                        for half in range(2):
                            dst = q_t[
                                half * 64 : half * 64 + 64,
                                h_abs * NQ : (h_abs + 1) * NQ,
                            ]
                            src = pqs[mi][lh * 64 : lh * 64 + 64, :]
                            if half == 0:
                                nc.vector.tensor_copy(dst, src)
                            else:
                                nc.scalar.copy(dst, src)

        # ---- attention-phase SBUF (proj pools released) ------------------
        with ExitStack() as att_ctx:
            apool = lambda name, bufs, **kw: att_ctx.enter_context(
                tc.tile_pool(name=name, bufs=bufs, **kw)
            )
            kv = apool("kv", 1)
            wo_pool = apool("wo", 1)
            es_pool = apool("es", 19)
            rp_pool = apool("rp", 2)
            y_pool = apool("y", 2)
            ps_s = apool("ps_s", 3, space="PSUM")

            # gathered K^T duplicated per head: quarter qb, head h at cols
            # (qb*16 + h)*512, with K_h^T in both row halves (see q_t note)
            kt_sb = kv.tile([128, GROUP * HEADS * NQ], MMDT)
            # gathered V interleaved with ones blocks: chunk kc spans
            # [kc*1600, +1600): pair c = h//2 at [c*192, +192) as
            # [ones | V_{2c} | V_{2c+1}], plus a trailing ones block.
            # Head h's lhsT = cols kc*1600 + c*192 + (h%2)*128, len 128:
            # even heads [ones | V] (AV rows 0:64 = colsum, 64:128 = data),
            # odd heads [V | ones] (swapped).
            vones = kv.tile([128, KCH * VSTR], MMDT)

            for kc in range(KCH):
                ones_base = vones[:, kc * VSTR : kc * VSTR + 64]
                nc.vector.memset(
                    bass.AP(
                        tensor=ones_base.tensor,
                        offset=ones_base.offset,
                        ap=[ones_base.ap[0], [192, DC + 1], [1, 64]],
                    ),
                    1.0,
                )

            # W_out chunks are DMA'd from inside the attention loop (on the
            # Sync queue, behind the later emit_loads) so they never compete
            # with the AllGathers or the latency-critical kt/vones loads.
            wout_sb = wo_pool.tile([128, DC * DIM], MMDT)

            def emit_wout(cs):
                for c in cs:
                    nc.sync.dma_start(
                        wout_sb[:, c * DIM : (c + 1) * DIM],
                        w_out[c * 128 : (c + 1) * 128, :],
                    )

            # per subgroup loads, interleaved per quarter (K then its V) so
            # the AV pipeline isn't starved behind all the K loads;
            # subgroup 0 now, later ones interleaved with the attention loop
            def emit_loads(s):
                hh = SG[s]
                nh = len(hh)
                vw = nh * 64
                for qb in range(GROUP):
                    # K^T: heads hh merged. Half 0 comes from HBM; half 1
                    # (the K=128 duplication) is an SBUF->SBUF copy so it
                    # doesn't compete with the AllGathers for HBM.
                    ksrc = bass.AP(
                        tensor=cc_outs[s].tensor,
                        offset=cc_outs[s].offset + qb * sg_len(hh),
                        ap=[[NQ, 64], [64 * NQ, nh], [1, NQ]],
                    )
                    span = slice(
                        (qb * HEADS + hh[0]) * NQ, (qb * HEADS + hh[0] + nh) * NQ
                    )
                    nc.sync.dma_start(kt_sb[0:64, span], ksrc)
                    # the K=128 duplication is an SBUF->SBUF DMA: no HBM
                    # traffic, and it cannot stall a compute engine's queue
                    nc.sync.dma_start(kt_sb[64:128, span], kt_sb[0:64, span])
                    # V: this quarter's 4 key chunks, one DMA per head pair
                    for pc in range(nh // 2):
                        vdst0 = vones[
                            :, qb * 4 * VSTR + (hh[0] // 2 + pc) * 192 + 64 :
                        ]
                        nc.sync.dma_start(
                            bass.AP(
                                tensor=vdst0.tensor,
                                offset=vdst0.offset,
                                ap=[vdst0.ap[0], [VSTR, 4], [1, 128]],
                            ),
                            bass.AP(
                                tensor=cc_outs[s].tensor,
                                offset=cc_outs[s].offset
                                + qb * sg_len(hh)
                                + sg_klen(hh)
                                + pc * 128,
                                ap=[[vw, 128], [128 * vw, 4], [1, 128]],
                            ),
                        )
            emit_loads(0)

            def av_lhs(h, kc):
                base = kc * VSTR + (h // 2) * 192 + (h % 2) * 128
                return vones[:, base : base + 128]

            # ---- attention: per head, 16 key chunks in pairs -------------
            # The first two heads of each head-group run "scores-ahead":
            # all S^T/exp pairs are emitted before any AV, so the PE/ACT
            # pipeline advances while the group's V loads drain.
            def head_scores(h):
                ess = []
                for pr in range(KPAIR):
                    pss = ps_s.tile([128, 1024], F32, tag="pss", name="pss")
                    for j in range(2):
                        kc = 2 * pr + j
                        qb, t4 = kc // 4, kc % 4
                        lhs_k = kt_sb[
                            :,
                            (qb * HEADS + h) * NQ + t4 * 128 : (qb * HEADS + h) * NQ
                            + (t4 + 1) * 128,
                        ]
                        nc.tensor.matmul(
                            pss[:, j * 512 : (j + 1) * 512],
                            lhs_k,
                            q_t[:, h * NQ : (h + 1) * NQ],
                            start=True,
                            stop=True,
                        )
                    es = es_pool.tile([128, 1024], MMDT, tag="es", name="es")
                    # psum holds 2*S (duplicated operands) -> halve the scale
                    nc.scalar.activation(es[:], pss[:], AF.Exp, scale=SCALE / 2)
                    ess.append(es)
                return ess

            def head_avs(h, ess):
                po = pp.tile([128, 512], F32, tag="acc", name="po")
                for pr in range(KPAIR):
                    for j in range(2):
                        kc = 2 * pr + j
                        nc.tensor.matmul(
                            po[:],
                            av_lhs(h, kc),
                            ess[pr][:, j * 512 : (j + 1) * 512],
                            start=(pr == 0 and j == 0),
                            stop=(pr == KPAIR - 1 and j == 1),
                        )
                return po

            def head_divide(h, po):
                hp = (h % 2) * 64
                hc = h // 2
                cb, dp = hp, 64 - hp
                recip = rp_pool.tile([128, 1024], F32, tag="recip", name="recip")
                nc.vector.tensor_copy(recip[0:64, 512:1024], po[cb : cb + 64, :])
                nc.vector.reciprocal_approx_fast(
                    recip[0:64, 0:512], recip[0:64, 512:1024]
                )
                nc.vector.tensor_mul(
                    att_t[hp : hp + 64, hc * NQ : (hc + 1) * NQ],
                    po[dp : dp + 64, :],
                    recip[0:64, 0:512],
                )

            def head_full(h):
                # normal interleaved S/exp/AV pipeline for one head
                po = pp.tile([128, 512], F32, tag="acc", name="po")
                for pr in range(KPAIR):
                    pss = ps_s.tile([128, 1024], F32, tag="pss", name="pss")
                    for j in range(2):
                        kc = 2 * pr + j
                        qb, t4 = kc // 4, kc % 4
                        lhs_k = kt_sb[
                            :,
                            (qb * HEADS + h) * NQ + t4 * 128 : (qb * HEADS + h)
                            * NQ
                            + (t4 + 1) * 128,
                        ]
                        nc.tensor.matmul(
                            pss[:, j * 512 : (j + 1) * 512],
                            lhs_k,
                            q_t[:, h * NQ : (h + 1) * NQ],
                            start=True,
                            stop=True,
                        )
                    es = es_pool.tile([128, 1024], MMDT, tag="es", name="es")
                    nc.scalar.activation(es[:], pss[:], AF.Exp, scale=SCALE / 2)
                    for j in range(2):
                        kc = 2 * pr + j
                        nc.tensor.matmul(
                            po[:],
                            av_lhs(h, kc),
                            es[:, j * 512 : (j + 1) * 512],
                            start=(pr == 0 and j == 0),
                            stop=(pr == KPAIR - 1 and j == 1),
                        )
                head_divide(h, po)

            wout_done = 0
            for s, hh in enumerate(SG):
                # first two heads run "scores-ahead", covering the V loads
                h0, h1 = hh[0], hh[1]
                ess0 = head_scores(h0)
                ess1 = head_scores(h1)
                po0 = head_avs(h0, ess0)
                head_divide(h0, po0)
                po1 = head_avs(h1, ess1)
                head_divide(h1, po1)
                for h in hh[2:]:
                    head_full(h)
                # next subgroup's loads go last: anything queued here that
                # waits on a not-yet-landed gather would stall this
                # subgroup's remaining work behind it
                if s + 1 < len(SG):
                    emit_loads(s + 1)
                if s >= 2 and wout_done < DC:
                    emit_wout([wout_done, wout_done + 1])
                    wout_done += 2

            # ---- output projection y^T = W_out^T @ att^T + b_out ---------
            # four interleaved accumulation chains (2x pp + halves of 2x
            # ps_s tiles — attention is done with them by now)
            for mp in range(DC // 4):
                pys = []
                for mi in range(4):
                    if mi < 2:
                        py = pp.tile([128, 512], F32, tag="acc", name=f"py{mi}")
                        pys.append(py[:])
                    else:
                        py = ps_s.tile([128, 1024], F32, tag="pss", name=f"py{mi}")
                        pys.append(py[:, 0:512])
                for c in range(DC):
                    for mi in range(4):
                        m = 4 * mp + mi
                        nc.tensor.matmul(
                            pys[mi],
                            wout_sb[:, c * DIM + m * 128 : c * DIM + (m + 1) * 128],
                            att_t[:, c * NQ : (c + 1) * NQ],
                            start=(c == 0),
                            stop=(c == DC - 1),
                        )
                for mi in range(4):
                    m = 4 * mp + mi
                    y_sb = y_pool.tile([128, 512], MMDT, tag="y")
                    nc.vector.tensor_scalar(
                        out=y_sb[:],
                        in0=pys[mi],
                        scalar1=gbb_t[:, 16 + m : 17 + m],
                        scalar2=None,
                        op0=ALU.add,
                    )
                    nc.sync.dma_start(out[m * 128 : (m + 1) * 128, :], y_sb[:])

    nc.compile()
    return nc


_NC_CACHE = None


def _get_nc():
    global _NC_CACHE
    if _NC_CACHE is None:
        _NC_CACHE = build_nc()
    return _NC_CACHE


def _make_in_maps(x, ln_gamma, ln_beta, W_qk, W_v, W_out, b_out):
    mmnp = mybir.dt.np(MMDT)
    wqk = np.asarray(W_qk, dtype=np.float32)
    wqk_q = np.ascontiguousarray(wqk[:, :INNER]).astype(mmnp)
    wqk_k = np.ascontiguousarray(wqk[:, INNER:]).astype(mmnp)
    wv = np.ascontiguousarray(np.asarray(W_v, dtype=np.float32)).astype(mmnp)
    wo = np.ascontiguousarray(np.asarray(W_out, dtype=np.float32)).astype(mmnp)
    gamma = np.asarray(ln_gamma, dtype=np.float32).reshape(DC, 128).T
    beta = np.asarray(ln_beta, dtype=np.float32).reshape(DC, 128).T
    bout = np.asarray(b_out, dtype=np.float32).reshape(DC, 128).T
    gbb = np.ascontiguousarray(np.concatenate([gamma, beta, bout], axis=1))
    xf = np.asarray(x, dtype=np.float32)
    in_maps = []
    for i in range(NCORES):
        g, qq = i // GROUP, i % GROUP
        in_maps.append(
            {
                "x": np.ascontiguousarray(xf[g, qq * NQ : (qq + 1) * NQ, :]),
                "gbb": gbb,
                "wqk_k": wqk_k,
                "wqk_q": wqk_q,
                "W_v": wv,
                "W_out": wo,
            }
        )
    return in_maps


def run(inputs: dict, trace: bool = False):
    """Run the distributed kernel; returns (full_output, BassKernelResults)."""
    nc = _get_nc()
    in_maps = _make_in_maps(**inputs)
    res = run_bass_kernel_spmd(
        nc, in_maps, core_ids=list(range(NCORES)), trace=trace
    )
    out_full = np.empty((B, N, DIM), dtype=np.float32)
    for i in range(NCORES):
        g, qq = i // GROUP, i % GROUP
        out_full[g, qq * NQ : (qq + 1) * NQ, :] = (
            res.results[i]["out"].astype(np.float32).T
        )
    return out_full, res


def kernel(**inputs) -> np.ndarray:
    out, _ = run(inputs, trace=False)
    return out



# revision 5
# speedup vs baseline: 1.0340x; 1.0340x over previous
"""Distributed Trainium2 Bass kernel for pre-LN multi-head attention (v2).

Reference computation (per batch b of 2, seq n=2048, dim=1024, 16 heads x 64):
    xn = LayerNorm(x) * gamma + beta
    q, k = split(xn @ W_qk); v = xn @ W_v
    out = softmax(q k^T / 8) v  (per head)
    y = out @ W_out + b_out

Sharding (v2, head-parallel attention): 8 cores = 2 batch groups x 4 head
groups. Core i owns batch g=i//4 and heads [4r, 4r+4) with r=i%4. Each core
receives the FULL batch x (host-cast to bf16), runs LayerNorm over all 2048
tokens (4x redundant but cheap vector work that hides under DMA), computes
K^T/V/Q projections for its own 4 heads over all 2048 tokens (same PE cycles
as the v1 token-sharded projections), and runs attention for its 4 heads over
all 2048 queries x 2048 keys. No mid-kernel K/V AllGather: v1 lost ~50us of
PE idle waiting on the serialized ~75GB/s collective stream that only starts
after the ~55us CC mesh bring-up barrier.

The only communication is a late attention-output exchange, when the CC mesh
is long up: attention output att^T is produced per (token-quarter, head-pair)
tile; one AllToAll redistributes quarter q's tiles to core q (AllToAll out
chunk r = rank r's chunk addressed to me = heads [4r,4r+4) of my quarter, so
the received buffer is exactly att^T [1024 inner, 512 tokens] in natural
inner-dim order). The exchange is split in two (head pairs 0-1 first, 2-3
at the end) so the first AllToAll hides under the second half of attention
and only the small second one sits near the tail. Each core then applies the
FULL W_out (contraction over all 16 heads) for its 512-token quarter and
writes y^T directly — host assembles 8 [1024, 512] shards, no reduction.

Attention-head loop ordering is h-outer / quarter-inner so head-pair 0-1
tiles for every quarter finish at the half-way point. The out-projection
accumulates even att chunks (delivered by AllToAll A) before odd ones so it
can start before AllToAll B lands.
"""
import sys
import types

sys.path.insert(0, "/opt/trn_rl_repo")

# Register the NTFF profile hook that trn_boot skips when the image's antenv
# lacks axon_hooks, so run_bass_kernel_spmd(trace=True) can report exec time.
if "antenv.axon_hooks" not in sys.modules:
    try:
        from trn_agent_boot.trn_boot import _ntff_profile_via_ctypes

        _hook = _ntff_profile_via_ctypes("/opt/axon/libaxon_pjrt.so")
    except Exception:
        _hook = None
    _mod = types.ModuleType("antenv.axon_hooks")
    _mod.get_axon_ntff_profile_hook = lambda: _hook
    _mod.set_axon_ntff_profile_hook = lambda h: None
    sys.modules["antenv.axon_hooks"] = _mod

from contextlib import ExitStack

import ml_dtypes
import numpy as np
import concourse.bass as bass
import concourse.tile as tile
from concourse import bacc, mybir
from concourse.bass_utils import run_bass_kernel_spmd
from concourse.masks import make_identity

F32 = mybir.dt.float32
BF16 = mybir.dt.bfloat16
AF = mybir.ActivationFunctionType
ALU = mybir.AluOpType

B, N, DIM = 2, 2048, 1024
HEADS, DH = 16, 64
INNER = HEADS * DH  # 1024
SCALE = DH**-0.5
EPS = 1e-5

NCORES = 8
GROUP = 4          # cores per batch group (head groups / output quarters)
LH = HEADS // GROUP  # 4 local heads per core
LIN = LH * DH      # 256 local inner dims
NQ = N // GROUP    # 512 tokens per output quarter
DC = DIM // 128    # 8 dim chunks
KCH = N // 128     # 16 key chunks of 128 tokens
KPAIR = KCH // 2   # exp batches of 2 key chunks
TG = 4             # token groups of 512 for LN/proj pipeline

MMDT = BF16        # matmul operand storage dtype

REPLICA_GROUPS = [[0, 1, 2, 3], [4, 5, 6, 7]]

VSTR = 448         # per-key-chunk vones stride: 2x[ones64|V_2c|V_2c+1] + ones64
CCSZ = GROUP * 128 * NQ  # one AllToAll buffer: 4 quarters x [128, 512]


def build_nc():
    nc = bacc.Bacc(num_devices=NCORES)

    x = nc.dram_tensor("x", [N, DIM], MMDT, kind="ExternalInput")
    gbb = nc.dram_tensor("gbb", [128, 24], F32, kind="ExternalInput")
    wqk_k = nc.dram_tensor("wqk_k", [DIM, LIN], MMDT, kind="ExternalInput")
    wqk_q = nc.dram_tensor("wqk_q", [DIM, LIN], MMDT, kind="ExternalInput")
    w_v = nc.dram_tensor("W_v", [DIM, LIN], MMDT, kind="ExternalInput")
    w_out = nc.dram_tensor("W_out", [INNER, DIM], MMDT, kind="ExternalInput")
    out = nc.dram_tensor("out", [DIM, NQ], MMDT, kind="ExternalOutput")

    with tile.TileContext(nc) as tc, ExitStack() as ctx:
        pool = lambda name, bufs, **kw: ctx.enter_context(
            tc.tile_pool(name=name, bufs=bufs, **kw)
        )
        consts = pool("consts", 1)
        dram = pool("dram", 1, space="DRAM")
        kv = pool("kv", 1)          # kt_sb + vones + q_t (live whole kernel)
        att_pool = pool("att", 1)
        small = pool("small", 8)
        pp = pool("pp", 2, space="PSUM")      # acc chains / AV accumulators

        # ---- constants ---------------------------------------------------
        gbb_t = consts.tile([128, 24], F32)   # [gamma | beta | b_out] per c
        nc.sync.dma_start(gbb_t[:], gbb[:, :])
        ident = consts.tile([128, 128], MMDT)
        make_identity(nc, ident[:])
        eps_sb = consts.tile([128, 1], F32)
        nc.vector.memset(eps_sb[:], EPS)
        # PE warmup: ramp the clock while input DMAs are in flight.
        wps = pp.tile([128, 512], F32, tag="acc", name="warmup")
        for i in range(24):
            nc.tensor.matmul(
                wps[:, 0:128], ident[:], ident[:], start=(i == 0), stop=(i == 23)
            )

        QSZ = 128 * NQ  # one staged quarter tile, flat
        cc_in_a = dram.tile([GROUP * QSZ], MMDT, name="cc_in_a")
        cc_out_a = dram.tile([GROUP * GROUP * QSZ], MMDT, name="cc_out_a")
        cc_in_b = dram.tile([GROUP * QSZ], MMDT, name="cc_in_b")
        cc_out_b = dram.tile([GROUP * GROUP * QSZ], MMDT, name="cc_out_b")

        # K^T per head, duplicated across both 64-row halves so the S^T
        # matmuls contract over K=128 (computing 2*S, folded into the exp
        # scale; K=64 matmuls were observed to hold the HAM clock at 1.2GHz).
        kt_sb = kv.tile([128, LH * N], MMDT)
        # V interleaved with ones blocks: key chunk kc spans [kc*448, +448):
        # [ones64 | V_h0 | V_h1 | ones64 | V_h2 | V_h3 | ones64]. Head h's AV
        # lhsT = cols kc*448 + (h//2)*192 + (h%2)*128, len 128: even heads
        # [ones | V] (AV rows 0:64 = colsum, 64:128 = data), odd swapped.
        vones = kv.tile([128, KCH * VSTR], MMDT)
        # Q^T duplicated per head like K^T (see kt_sb note).
        q_t = kv.tile([128, LH * N], MMDT)
        # attention output^T: quarter q, head pair hc at chunk (q*2 + hc),
        # [128, 512] with head parity on the 64-row halves.
        att_t = att_pool.tile([128, GROUP * 2 * NQ], MMDT)

        for kc in range(KCH):
            ones_base = vones[:, kc * VSTR : kc * VSTR + 64]
            nc.vector.memset(
                bass.AP(
                    tensor=ones_base.tensor,
                    offset=ones_base.offset,
                    ap=[ones_base.ap[0], [192, 3], [1, 64]],
                ),
                1.0,
            )

        with ExitStack() as proj_ctx:
            ppool = lambda name, bufs, **kw: proj_ctx.enter_context(
                tc.tile_pool(name=name, bufs=bufs, **kw)
            )
            ptr = ppool("ptr", 2, space="PSUM")  # transpose targets
            pkv = ppool("pkv", 2, space="PSUM")  # second accumulation chain
            xw = ppool("xw", 1)
            xn_pool = ppool("xn", 2)
            x_sb = xw.tile([128, (N // 128) * DIM], MMDT)
            xnt = xw.tile([128, DC * N], MMDT)
            wk_sb = xw.tile([128, DC * LIN], MMDT)
            wv_sb = xw.tile([128, DC * LIN], MMDT)
            wq_sb = xw.tile([128, DC * LIN], MMDT)

            # Input loads in consumption order, all on the Sync queue,
            # <=256KB per DMA.
            for t in range(N // 128):
                nc.sync.dma_start(
                    x_sb[:, t * DIM : (t + 1) * DIM],
                    x[t * 128 : (t + 1) * 128, :],
                )
            for w_sb, w_hbm in ((wk_sb, wqk_k), (wv_sb, w_v), (wq_sb, wqk_q)):
                for c in range(DC):
                    nc.sync.dma_start(
                        w_sb[:, c * LIN : (c + 1) * LIN],
                        w_hbm[c * 128 : (c + 1) * 128, :],
                    )

            # ---- LayerNorm + transpose, per group of 4 token chunks ------
            def ln_group(tg):
                xn_t = xn_pool.tile([128, 4 * DIM], MMDT, tag="xn")
                for i in range(4):
                    t = tg * 4 + i
                    xt = x_sb[:, t * DIM : (t + 1) * DIM]
                    xg = xt.rearrange("p (n s) -> p n s", s=512)
                    stats = small.tile([128, 2, 6], F32)
                    for sgi in range(2):
                        nc.vector.bn_stats(stats[:, sgi, :], xg[:, sgi, :])
                    mv = small.tile([128, 2], F32)
                    nc.vector.bn_aggr(mv[:], stats[:])
                    rstd = small.tile([128, 1], F32)
                    nc.scalar.activation(rstd[:], mv[:, 1:2], AF.Sqrt, bias=eps_sb[:])
                    nc.vector.reciprocal(rstd[:], rstd[:])
                    nc.vector.tensor_scalar(
                        out=xn_t[:, i * DIM : (i + 1) * DIM],
                        in0=xt,
                        scalar1=mv[:, 0:1],
                        scalar2=rstd[:],
                        op0=ALU.subtract,
                        op1=ALU.mult,
                    )
                # transpose to [dim, tokens], fusing gamma/beta; split the
                # scale/cast between Vector and Scalar engines
                for c in range(DC):
                    pt = ptr.tile([128, 512], MMDT, tag="tr")
                    for i in range(4):
                        nc.tensor.transpose(
                            pt[:, i * 128 : (i + 1) * 128],
                            xn_t[:, i * DIM + c * 128 : i * DIM + (c + 1) * 128],
                            ident[:],
                        )
                    dst = xnt[:, c * N + tg * 512 : c * N + (tg + 1) * 512]
                    if (c + tg) % 2 == 0:
                        nc.vector.tensor_scalar(
                            out=dst,
                            in0=pt[:],
                            scalar1=gbb_t[:, c : c + 1],
                            scalar2=gbb_t[:, 8 + c : 9 + c],
                            op0=ALU.mult,
                            op1=ALU.add,
                        )
                    else:
                        nc.scalar.activation(
                            dst,
                            pt[:],
                            AF.Identity,
                            bias=gbb_t[:, 8 + c : 9 + c],
                            scale=gbb_t[:, c : c + 1],
                        )

            # ---- K^T / Q^T projection for token span s (2 chains) --------
            def kq_proj(w_sb, dst, s):
                pqs = [
                    (pp if m == 0 else pkv).tile(
                        [128, 512], F32, tag="acc" if m == 0 else "kvacc",
                        name=f"pkq{m}",
                    )
                    for m in range(2)
                ]
                for c in range(DC):
                    for m in range(2):
                        nc.tensor.matmul(
                            pqs[m][:],
                            w_sb[:, c * LIN + m * 128 : c * LIN + (m + 1) * 128],
                            xnt[:, c * N + s * 512 : c * N + (s + 1) * 512],
                            start=(c == 0),
                            stop=(c == DC - 1),
                        )
                for m in range(2):
                    for lh in range(2):
                        h = 2 * m + lh
                        span = slice(h * N + s * 512, h * N + (s + 1) * 512)
                        src = pqs[m][lh * 64 : lh * 64 + 64, :]
                        if (m + lh) % 2 == 0:
                            nc.vector.tensor_copy(dst[0:64, span], src)
                        else:
                            nc.scalar.copy(dst[0:64, span], src)
                        # K=128 duplication: SBUF->SBUF DMA, no HBM traffic
                        nc.sync.dma_start(dst[64:128, span], dst[0:64, span])

            # ---- V projection for token span s (2+2 chains) --------------
            def v_proj(s):
                for tp in range(2):
                    pvs = [
                        (pp if ti == 0 else pkv).tile(
                            [128, 512], F32, tag="acc" if ti == 0 else "kvacc",
                            name=f"pv{ti}",
                        )
                        for ti in range(2)
                    ]
                    for c in range(DC):
                        for ti in range(2):
                            t = s * 4 + 2 * tp + ti
                            nc.tensor.matmul(
                                pvs[ti][:, 0:LIN],
                                xnt[:, c * N + t * 128 : c * N + (t + 1) * 128],
                                wv_sb[:, c * LIN : (c + 1) * LIN],
                                start=(c == 0),
                                stop=(c == DC - 1),
                            )
                    for ti in range(2):
                        kc = s * 4 + 2 * tp + ti
                        vdst = vones[:, kc * VSTR + 64 :]
                        dst_ap = bass.AP(
                            tensor=vdst.tensor,
                            offset=vdst.offset,
                            ap=[vdst.ap[0], [192, 2], [1, 128]],
                        )
                        src = pvs[ti][:, 0:LIN].rearrange(
                            "p (n f) -> p n f", f=128
                        )
                        if ti == 0:
                            nc.vector.tensor_copy(dst_ap, src)
                        else:
                            nc.scalar.copy(dst_ap, src)

            for tg in range(TG):
                ln_group(tg)
            for s in range(TG):
                kq_proj(wk_sb, kt_sb, s)
            for s in range(TG):
                v_proj(s)
            # Q: interleave two spans per head-pair chunk m, m=0 first so
            # heads 0-1 are ready when attention starts.
            for m in range(2):
                for sp in range(2):
                    pqs = []
                    for si in range(2):
                        pq = (pp if si == 0 else pkv).tile(
                            [128, 512], F32, tag="acc" if si == 0 else "kvacc",
                            name=f"pq{si}",
                        )
                        pqs.append(pq)
                    for c in range(DC):
                        for si in range(2):
                            s = 2 * sp + si
                            nc.tensor.matmul(
                                pqs[si][:],
                                wq_sb[:, c * LIN + m * 128 : c * LIN + (m + 1) * 128],
                                xnt[:, c * N + s * 512 : c * N + (s + 1) * 512],
                                start=(c == 0),
                                stop=(c == DC - 1),
                            )
                    for si in range(2):
                        s = 2 * sp + si
                        for lh in range(2):
                            h = 2 * m + lh
                            span = slice(h * N + s * 512, h * N + (s + 1) * 512)
                            src = pqs[si][lh * 64 : lh * 64 + 64, :]
                            if (si + lh) % 2 == 0:
                                nc.vector.tensor_copy(q_t[0:64, span], src)
                            else:
                                nc.scalar.copy(q_t[0:64, span], src)
                            nc.sync.dma_start(q_t[64:128, span], q_t[0:64, span])

        # ---- attention-phase SBUF (proj pools released) ------------------
        with ExitStack() as att_ctx:
            apool = lambda name, bufs, **kw: att_ctx.enter_context(
                tc.tile_pool(name=name, bufs=bufs, **kw)
            )
            wo_pool = apool("wo", 1)
            es_pool = apool("es", 8)
            rp_pool = apool("rp", 2)
            y_pool = apool("y", 2)
            ps_s = apool("ps_s", 3, space="PSUM")

            wout_sb = wo_pool.tile([128, DC * DIM], MMDT)
            att_full = wo_pool.tile([128, DC * NQ], MMDT)

            def emit_wout(cs):
                for c in cs:
                    nc.sync.dma_start(
                        wout_sb[:, c * DIM : (c + 1) * DIM],
                        w_out[c * 128 : (c + 1) * 128, :],
                    )

            def av_lhs(h, kc):
                base = kc * VSTR + (h // 2) * 192 + (h % 2) * 128
                return vones[:, base : base + 128]

            def head_divide(h, qq, po):
                hp = (h % 2) * 64
                hc = h // 2
                cb, dp = hp, 64 - hp
                recip = rp_pool.tile([128, 1024], F32, tag="recip", name="recip")
                nc.vector.tensor_copy(recip[0:64, 512:1024], po[cb : cb + 64, :])
                nc.vector.reciprocal_approx_fast(
                    recip[0:64, 0:512], recip[0:64, 512:1024]
                )
                nc.vector.tensor_mul(
                    att_t[hp : hp + 64, (qq * 2 + hc) * NQ : (qq * 2 + hc + 1) * NQ],
                    po[dp : dp + 64, :],
                    recip[0:64, 0:512],
                )

            def head_unit(h, qq):
                # interleaved S/exp/AV pipeline for one (head, quarter)
                po = pp.tile([128, 512], F32, tag="acc", name="po")
                for pr in range(KPAIR):
                    pss = ps_s.tile([128, 1024], F32, tag="pss", name="pss")
                    for j in range(2):
                        kc = 2 * pr + j
                        nc.tensor.matmul(
                            pss[:, j * 512 : (j + 1) * 512],
                            kt_sb[:, h * N + kc * 128 : h * N + (kc + 1) * 128],
                            q_t[:, h * N + qq * 512 : h * N + (qq + 1) * 512],
                            start=True,
                            stop=True,
                        )
                    es = es_pool.tile([128, 1024], MMDT, tag="es", name="es")
                    # psum holds 2*S (duplicated operands) -> halve the scale
                    nc.scalar.activation(es[:], pss[:], AF.Exp, scale=SCALE / 2)
                    for j in range(2):
                        kc = 2 * pr + j
                        nc.tensor.matmul(
                            po[:],
                            av_lhs(h, kc),
                            es[:, j * 512 : (j + 1) * 512],
                            start=(pr == 0 and j == 0),
                            stop=(pr == KPAIR - 1 and j == 1),
                        )
                head_divide(h, qq, po)

            QSZ = 128 * NQ
            rid = nc.engines[mybir.EngineType.SP].partition_id() % GROUP

            def stage_and_gather(cc_in, cc_out, qq, lc):
                # stage quarter qq's head-pair-lc tile, then AllGather it
                # within the batch group (fires as soon as the stage lands)
                nc.sync.dma_start(
                    cc_in[qq * QSZ : (qq + 1) * QSZ].rearrange(
                        "(p f) -> p f", f=NQ
                    ),
                    att_t[:, (qq * 2 + lc) * NQ : (qq * 2 + lc + 1) * NQ],
                )
                nc.gpsimd.collective_compute(
                    "AllGather",
                    ALU.bypass,
                    replica_groups=REPLICA_GROUPS,
                    ins=[cc_in[qq * QSZ : (qq + 1) * QSZ].opt()],
                    outs=[
                        cc_out[
                            qq * GROUP * QSZ : (qq + 1) * GROUP * QSZ
                        ].opt()
                    ],
                )

            def load_att(cc_out, lc):
                # my quarter's gathered block: rank r's slice = heads
                # [4r, 4r+4) pair lc -> att_full chunk c = 2r + lc. One DMA
                # with a runtime (rank-dependent) source offset.
                blk = cc_out[bass.ds(rid * GROUP * QSZ, GROUP * QSZ)]
                src = bass.AP(
                    tensor=blk.tensor,
                    offset=blk.offset,
                    ap=[[NQ, 128], [QSZ, GROUP], [1, NQ]],
                )
                dst0 = att_full[:, lc * NQ :]
                dst = bass.AP(
                    tensor=dst0.tensor,
                    offset=dst0.offset,
                    ap=[dst0.ap[0], [2 * NQ, GROUP], [1, NQ]],
                )
                nc.sync.dma_start(dst, src)

            # h-outer: head pairs 0-1 finish all quarters halfway through,
            # so the A exchange hides under the second half of attention.
            for h in range(2):
                for qq in range(GROUP):
                    head_unit(h, qq)
                    if h == 1:
                        stage_and_gather(cc_in_a, cc_out_a, qq, 0)
            load_att(cc_out_a, 0)
            for h in range(2, LH):
                for qq in range(GROUP):
                    head_unit(h, qq)
                    if h == 2 and qq % 2 == 1:
                        emit_wout(range((qq // 2) * 4, (qq // 2) * 4 + 4))
                    if h == LH - 1:
                        stage_and_gather(cc_in_b, cc_out_b, qq, 1)
            load_att(cc_out_b, 1)

            # ---- output projection y^T = W_out^T @ att^T + b_out ---------
            # four interleaved accumulation chains; even att chunks (from
            # AllToAll A) accumulate first so work starts before B lands
            C_ORDER = [0, 2, 4, 6, 1, 3, 5, 7]
            for mp in range(DC // 4):
                pys = []
                for mi in range(4):
                    if mi < 2:
                        py = pp.tile([128, 512], F32, tag="acc", name=f"py{mi}")
                        pys.append(py[:])
                    else:
                        py = ps_s.tile([128, 1024], F32, tag="pss", name=f"py{mi}")
                        pys.append(py[:, 0:512])
                for ci, c in enumerate(C_ORDER):
                    for mi in range(4):
                        m = 4 * mp + mi
                        nc.tensor.matmul(
                            pys[mi],
                            wout_sb[:, c * DIM + m * 128 : c * DIM + (m + 1) * 128],
                            att_full[:, c * NQ : (c + 1) * NQ],
                            start=(ci == 0),
                            stop=(ci == DC - 1),
                        )
                for mi in range(4):
                    m = 4 * mp + mi
                    y_sb = y_pool.tile([128, 512], MMDT, tag="y")
                    nc.vector.tensor_scalar(
                        out=y_sb[:],
                        in0=pys[mi],
                        scalar1=gbb_t[:, 16 + m : 17 + m],
                        scalar2=None,
                        op0=ALU.add,
                    )
                    nc.sync.dma_start(out[m * 128 : (m + 1) * 128, :], y_sb[:])

    nc.compile()
    return nc


_NC_CACHE = None


def _get_nc():
    global _NC_CACHE
    if _NC_CACHE is None:
        _NC_CACHE = build_nc()
    return _NC_CACHE


def _make_in_maps(x, ln_gamma, ln_beta, W_qk, W_v, W_out, b_out):
    mmnp = mybir.dt.np(MMDT)
    wqk = np.asarray(W_qk, dtype=np.float32)
    wv = np.asarray(W_v, dtype=np.float32)
    wo = np.ascontiguousarray(np.asarray(W_out, dtype=np.float32)).astype(mmnp)
    gamma = np.asarray(ln_gamma, dtype=np.float32).reshape(DC, 128).T
    beta = np.asarray(ln_beta, dtype=np.float32).reshape(DC, 128).T
    bout = np.asarray(b_out, dtype=np.float32).reshape(DC, 128).T
    gbb = np.ascontiguousarray(np.concatenate([gamma, beta, bout], axis=1))
    xf = np.asarray(x, dtype=np.float32)
    xb = [np.ascontiguousarray(xf[g]).astype(mmnp) for g in range(B)]
    in_maps = []
    for i in range(NCORES):
        g, r = i // GROUP, i % GROUP
        cols = slice(r * LIN, (r + 1) * LIN)
        kcols = slice(INNER + r * LIN, INNER + (r + 1) * LIN)
        in_maps.append(
            {
                "x": xb[g],
                "gbb": gbb,
                "wqk_k": np.ascontiguousarray(wqk[:, kcols]).astype(mmnp),
                "wqk_q": np.ascontiguousarray(wqk[:, cols]).astype(mmnp),
                "W_v": np.ascontiguousarray(wv[:, cols]).astype(mmnp),
                "W_out": wo,
            }
        )
    return in_maps


def run(inputs: dict, trace: bool = False):
    """Run the distributed kernel; returns (full_output, BassKernelResults)."""
    nc = _get_nc()
    in_maps = _make_in_maps(**inputs)
    res = run_bass_kernel_spmd(
        nc, in_maps, core_ids=list(range(NCORES)), trace=trace
    )
    out_full = np.empty((B, N, DIM), dtype=np.float32)
    for i in range(NCORES):
        g, r = i // GROUP, i % GROUP
        out_full[g, r * NQ : (r + 1) * NQ, :] = (
            res.results[i]["out"].astype(np.float32).T
        )
    return out_full, res


def kernel(**inputs) -> np.ndarray:
    out, _ = run(inputs, trace=False)
    return out
